# revision 4
# baseline (speedup 1.0000x reference)
"""CopyGenerator kernel for 8 trn2 NeuronCores.

Strategy (vocab tensor-parallel, fp8 DoubleRow):
  - W's vocab dim (50000) is sharded 6250 cols/core, padded to 6272, scaled
    by 32 on the host and kept fp8e4 FULLY RESIDENT in SBUF (49KB/partition,
    loaded once at kernel start as 13 column-group DMAs).
  - Per core: logits = hidden @ (32*W_shard) via PE in fp8e4 with
    perf_mode=DoubleRow (two 128-deep k-slices per matmul, 2x fp8 rate),
    fp32 PSUM accumulate over 4 k-pairs. exp via ACT with scale=1/32 and
    fused row-sum accumulation (exp kept fp16 in SBUF). Softmax denominator
    completed with one tiny AllReduce per row-chunk, then an in-place DVE
    scale applies (1-copy)/Z and each row tile is stored as one big fp16 DMA.
  - Rows processed in 5 chunks (4,4,4,2,2 n-tiles of 128 rows): chunk i+1's
    matmuls hide chunk i's AllReduce latency; the small final chunks keep the
    un-overlapped tail (last AllReduce + scale + store) short.
  - The copy gate sigmoid(hidden@w_copy) is folded in on the host: the host
    passes (1-c) transposed per-partition and attn*c pre-multiplied, so the
    device only runs the einsum vs src_map (32 small fp16 matmuls), emitted
    at the END where they hide inside the final AllReduce/store tail.
  - Queue discipline (queues are in-order): W + copy-path stores on sync,
    hT/exp/small loads on scalar, AllReduce chain (zin, collective, zr) and
    main stores on gpsimd so a waiting zr load never blocks an hT prefetch.
PAD col handling: host zeroes W[:,1] on core 0, kernel masks the exp column
and subtracts the constant exp(0)=1 from that core's row sums.
Output is fp16 on device; host upcasts to fp32.
"""

import numpy as np

N, D, V = 2048, 1024, 50000
S, B, CV = 100, 32, 120
NCORES = 8
VREAL = 6250          # real vocab cols per core
VPAD = 6272           # padded (12*512 + 128)
KT = 8                # k-tiles of 128 over D
JP = KT // 2          # DoubleRow k-pairs
NT = N // 128         # 16 row tiles
CHUNKS = [(0, 4), (4, 4), (8, 4), (12, 2), (14, 2)]   # (first n-tile, count)
# v-tiles: (col offset, matmul width, exp width)
GTILES = [(i * 512, 512, 512) for i in range(12)] + [(6144, 128, 106)]
PAD_IDX = 1
WSCALE = 32.0

_CACHE = {}
TRACE = False


def _install_walrus_compat():
    """This container's walrus build rejects >1 sync-wait per instruction.
    Patch the Tile drain to chain single-wait drains, and provide a module
    post-pass hoisting extra waits onto same-engine NoOps."""
    import concourse.tile as tile_mod
    import concourse.mybir as mybir
    from concourse.vector_clock import ScopedClock

    if getattr(tile_mod.TileContext._drain_and_barrier, "_waitsplit", False):
        return

    def _patched_drain_and_barrier(self, tick_clock, wait_clock):
        nc = self.nc
        drain_inst = nc.sync.drain()
        wait_clock.add_sem_waits(
            drain_inst.ins, ScopedClock({None: tick_clock.global_clock})
        )
        si = drain_inst.ins.sync_info
        waits = list(si.on_wait) if si and si.on_wait else []
        if len(waits) > 1:
            si.on_wait = waits[:1]
            rest = waits[1:]
            while rest:
                chunk, rest = rest[:1], rest[1:]
                d2 = nc.sync.drain()
                if d2.ins.sync_info is None:
                    d2.ins.sync_info = mybir.SyncInfo(on_wait=chunk, on_update=[])
                else:
                    d2.ins.sync_info.on_wait = chunk
        nc.all_engine_barrier()
        assert self.sems is not None
        popped = nc._tile_sem_poison_stack.pop()
        assert popped is self._sem_poison
        nc.clear_and_free_semaphores(list(self.sems.allocated().values()))
        nc.all_engine_barrier()

    _patched_drain_and_barrier._waitsplit = True
    tile_mod.TileContext._drain_and_barrier = _patched_drain_and_barrier


def _split_multi_waits(nc):
    import concourse.mybir as mybir

    uid = 0
    n_split = 0
    for fn in nc.m.functions:
        for bb in fn.blocks:
            old = list(bb.instructions)
            new = []
            changed = False
            for ins in old:
                si = ins.sync_info
                waits = list(si.on_wait) if si and si.on_wait else []
                if len(waits) > 1:
                    changed = True
                    n_split += 1
                    for w in waits[:-1]:
                        uid += 1
                        new.append(
                            mybir.InstNoOp(
                                name=f"I-waitsplit-{uid}-{ins.name}",
                                sync_info=mybir.SyncInfo(on_wait=[w], on_update=[]),
                                bass_nofuse=True,
                                engine=ins.engine,
                            )
                        )
                    si.on_wait = [waits[-1]]
                new.append(ins)
            if changed:
                bb.instructions[:] = new
    return n_split


def _build_nc():
    import concourse.bass as bass
    import concourse.mybir as mybir
    import concourse.tile as tile

    _install_walrus_compat()

    f32 = mybir.dt.float32
    f16 = mybir.dt.float16
    f8 = mybir.dt.float8e4
    AF = mybir.ActivationFunctionType
    OP = mybir.AluOpType
    AX = mybir.AxisListType
    DR = mybir.MatmulPerfMode.DoubleRow

    nc = bass.Bass()
    hT8 = nc.dram_tensor("hT8", [D, N], f8, kind="ExternalInput")
    W8 = nc.dram_tensor("W8", [D, VPAD], f8, kind="ExternalInput")
    mulT16 = nc.dram_tensor("mulT16", [S, N], f16, kind="ExternalInput")
    smap = nc.dram_tensor("smap", [S, B * CV], f16, kind="ExternalInput")
    omcT = nc.dram_tensor("omcT", [128, NT], f32, kind="ExternalInput")
    cmask = nc.dram_tensor("cmask", [128, 2], f16, kind="ExternalInput")
    zcorr = nc.dram_tensor("zcorr", [128, 1], f32, kind="ExternalInput")
    out = nc.dram_tensor("out", [N, VREAL + CV], f16, kind="ExternalOutput")

    hT8_r = hT8.rearrange("(k p) n -> p k n", p=128)
    W8_r = W8.rearrange("(k p) v -> p k v", p=128)
    MAXNT = max(cnt for _, cnt in CHUNKS)

    with tile.TileContext(nc) as tc:
        with (
            tc.tile_pool(name="htp", bufs=2) as htp,
            tc.tile_pool(name="expp", bufs=8) as expp,
            tc.tile_pool(name="zpp", bufs=2 * MAXNT) as zpp,
            tc.tile_pool(name="cpop", bufs=4) as cpop,
            tc.tile_pool(name="smallp", bufs=1) as smallp,
            tc.tile_pool(name="psmain", bufs=7, space="PSUM") as psmain,
            tc.tile_pool(name="psaux", bufs=1, space="PSUM") as psaux,
            tc.tile_pool(name="dramp", bufs=1, space="DRAM") as dramp,
        ):
            # ---- resident W shard: 13 column-group loads on sync queue ----
            wres = smallp.tile([128, KT, VPAD], f8)
            for goff, gw, _ in GTILES:
                nc.sync.dma_start(
                    wres[:, :, goff : goff + gw], W8_r[0:128, 0:KT, goff : goff + gw]
                )

            # ---- persistent small tiles (scalar-queue loads) ----
            cmask_sb = smallp.tile([128, 2], f16)
            nc.scalar.dma_start(cmask_sb[:], cmask[:])
            zcorr_sb = smallp.tile([128, 1], f32)
            nc.scalar.dma_start(zcorr_sb[:], zcorr[:])
            omcT_sb = smallp.tile([128, NT], f32)
            nc.scalar.dma_start(omcT_sb[:], omcT[:])
            mulT_sb = smallp.tile([128, N], f16)
            smap_sb = smallp.tile([128, B * CV], f16)

            zin = [
                dramp.tile([128, cnt], f32, name=f"zin{ci}")
                for ci, (_, cnt) in enumerate(CHUNKS)
            ]
            zout = [
                dramp.tile([128, cnt], f32, addr_space="Shared", name=f"zout{ci}")
                for ci, (_, cnt) in enumerate(CHUNKS)
            ]

            # ---- main chunks ----
            for ci, (t0, NTC) in enumerate(CHUNKS):
                ncols = NTC * 128          # rows of this chunk
                c0 = t0 * 128              # first row
                # hidden^T chunk: [p, k, n] layout for DoubleRow pairs
                htc = htp.tile([128, KT, MAXNT * 128], f8, tag="ht", name=f"ht{ci}")
                nc.scalar.dma_start(
                    htc[:, :, 0:ncols], hT8_r[0:128, 0:KT, c0 : c0 + ncols]
                )
                if ci == 1:
                    # copy-path inputs: load mid-program, used at the end
                    nc.scalar.dma_start(mulT_sb[0:S, :], mulT16[:, :])
                    nc.scalar.dma_start(smap_sb[0:S, :], smap[:, :])

                exps = [
                    expp.tile([128, VREAL], f16, tag="exp", name=f"exp{ci}_{t}")
                    for t in range(NTC)
                ]
                zparts = [
                    zpp.tile([128, len(GTILES)], f32, tag="zpart", name=f"zp{ci}_{t}")
                    for t in range(NTC)
                ]
                for gi, (goff, gw, ew) in enumerate(GTILES):
                    for t in range(NTC):
                        pm = psmain.tile(
                            [128, 512], f32, tag="psmain", name=f"pm{ci}_{gi}_{t}"
                        )
                        for j in range(JP):
                            nc.tensor.matmul(
                                pm[:, 0:gw],
                                htc[:, 2 * j : 2 * j + 2, t * 128 : (t + 1) * 128],
                                wres[:, 2 * j : 2 * j + 2, goff : goff + gw],
                                start=(j == 0),
                                stop=(j == JP - 1),
                                perf_mode=DR,
                            )
                        nc.scalar.activation(
                            exps[t][:, goff : goff + ew], pm[:, 0:ew], AF.Exp,
                            scale=1.0 / WSCALE,
                            accum_out=zparts[t][:, gi : gi + 1],
                        )
                        if gi == 0:
                            # zero masked cols (PAD on core 0; all-ones elsewhere)
                            nc.vector.tensor_tensor(
                                exps[t][:, 0:2], exps[t][:, 0:2], cmask_sb[:],
                                OP.mult,
                            )

                # ---- denominator: reduce partials, AllReduce across cores ----
                zsum = smallp.tile([128, NTC], f32, name=f"zsum{ci}")
                for t in range(NTC):
                    nc.vector.tensor_reduce(
                        zsum[:, t : t + 1], zparts[t][:, :], axis=AX.X, op=OP.add
                    )
                nc.vector.tensor_scalar(
                    zsum[:], zsum[:], zcorr_sb[:], None, OP.subtract
                )
                nc.gpsimd.dma_start(zin[ci][:], zsum[:])
                nc.gpsimd.collective_compute(
                    "AllReduce",
                    OP.add,
                    ins=[zin[ci].opt()],
                    outs=[zout[ci].opt()],
                    replica_groups=[list(range(NCORES))],
                )
                zr = smallp.tile([128, NTC], f32, name=f"zr{ci}")
                nc.gpsimd.dma_start(zr[:], zout[ci][:])
                rz = smallp.tile([128, NTC], f32, name=f"rz{ci}")
                nc.vector.reciprocal(rz[:], zr[:])
                sc = smallp.tile([128, NTC], f32, name=f"sc{ci}")
                nc.vector.tensor_tensor(
                    sc[:], omcT_sb[:, t0 : t0 + NTC], rz[:], OP.mult
                )

                # ---- pass 2: in-place scale on DVE, one big store per tile ----
                for t in range(NTC):
                    r0 = (t0 + t) * 128
                    nc.vector.tensor_scalar(
                        exps[t][:, 0:VREAL],
                        exps[t][:, 0:VREAL],
                        sc[:, t : t + 1],
                        None,
                        OP.mult,
                    )
                    nc.gpsimd.dma_start(
                        out[r0 : r0 + 128, 0:VREAL], exps[t][:, 0:VREAL]
                    )

            # ---- copy path last: copy_prob = einsum(attn*copy, src_map) ----
            # (hides inside the final AllReduce + store tail)
            mulT_r = mulT_sb.rearrange("p (t b) -> p b t", b=B)
            out_r = out[:, :].rearrange("(t b) v -> b t v", b=B)
            for bb_ in range(B):
                pc = psaux.tile([64, CV], f32, tag="psaux", name=f"pc{bb_}")
                nc.tensor.matmul(
                    pc[:],
                    mulT_r[0:S, bb_, :],
                    smap_sb[0:S, bb_ * CV : (bb_ + 1) * CV],
                    start=True,
                    stop=True,
                )
                cpo = cpop.tile([64, CV], f16, tag="cpo", name=f"cpo{bb_}")
                nc.vector.tensor_copy(cpo[:], pc[:])
                nc.sync.dma_start(out_r[bb_, :, VREAL : VREAL + CV], cpo[:])

    _split_multi_waits(nc)
    return nc


def _get_nc():
    if "nc" not in _CACHE:
        _CACHE["nc"] = _build_nc()
    return _CACHE["nc"]


def kernel(**inputs):
    import ml_dtypes
    from concourse.bass_utils import run_bass_kernel_spmd

    f8np = ml_dtypes.float8_e4m3

    hidden = np.asarray(inputs["hidden"], np.float32)
    attn = np.asarray(inputs["attn"], np.float32)
    src_map = np.asarray(inputs["src_map"], np.float32)
    W = np.asarray(inputs["W"], np.float32)
    w_copy = np.asarray(inputs["w_copy"], np.float32)
    b_copy = np.asarray(inputs["b_copy"], np.float32)

    nc = _get_nc()

    # host-side copy gate (tiny: N x D @ D x 1)
    c = 1.0 / (1.0 + np.exp(-(hidden @ w_copy + b_copy)))      # [N, 1] f32
    omc = (1.0 - c[:, 0]).astype(np.float32)                   # [N]
    omcT_h = np.ascontiguousarray(omc.reshape(NT, 128).T)      # [128, NT]
    mulT_h = np.ascontiguousarray((attn * c).T).astype(np.float16)  # [S, N]

    hT8_h = np.ascontiguousarray(hidden.T).astype(f8np)        # [D, N]
    smap16 = np.ascontiguousarray(src_map.reshape(S, B * CV)).astype(np.float16)

    in_maps = []
    for cc in range(NCORES):
        Wc = np.zeros((D, VPAD), f8np)
        Wcf = W[:, cc * VREAL : (cc + 1) * VREAL] * WSCALE
        if cc == 0:
            Wcf = Wcf.copy()
            Wcf[:, PAD_IDX] = 0.0
        Wc[:, :VREAL] = Wcf.astype(f8np)
        cm = np.ones((128, 2), np.float16)
        zc = np.zeros((128, 1), np.float32)
        if cc == 0:
            cm[:, PAD_IDX] = 0.0
            zc[:] = 1.0
        in_maps.append(
            {
                "hT8": hT8_h,
                "W8": Wc,
                "mulT16": mulT_h,
                "smap": smap16,
                "omcT": omcT_h,
                "cmask": cm,
                "zcorr": zc,
            }
        )

    res = run_bass_kernel_spmd(nc, in_maps, list(range(NCORES)), trace=TRACE)
    _CACHE["last_result"] = res

    outs = [r["out"] for r in res.results]
    full = np.empty((N, V + CV), np.float32)
    for cc in range(NCORES):
        full[:, cc * VREAL : (cc + 1) * VREAL] = outs[cc][:, :VREAL]
    full[:, V:] = outs[0][:, VREAL:]
    return full
